# revision 20
# baseline (speedup 1.0000x reference)
"""DigitCaps dynamic-routing kernel for 8 Trainium2 NeuronCores.

Strategy: shard the ROUTE dimension (4608 -> 576/core) instead of batch.
Per core, the x-shard and the packed W-shard fit in SBUF (bf16: 3.5MB), so
all 5 passes over u_hat (3x s-pass, 2x a-pass) recompute u_hat from
SBUF-resident data via PE matmuls -- no HBM re-streaming.  The only
cross-core traffic is one fused AllReduce per routing iteration carrying
s_unnorm [128,256] + the local softmax denominator [16].  The routing
logits b stay core-local (they are route-indexed).

Precision: matmul operands for the two big passes are bf16 (fp32 PSUM
accumulation); everything else fp32.  Validated end-to-end rel err ~2.6e-3
vs the fp32 reference (numpy bf16 pipeline study).

Math mapping (per core, r0 = core*576, 36 blocks of 16 routes):
  xt   [128b, (r,i)=4608]          x[:, r0:r0+576, :] flattened (bf16)
  xpk_j [128=(rl,i), 128b]         DMA-transposed block of xt (bf16)
  wp_j [128=(rl,i), 256=(c,o)]     W[r0+16j+rl, c, o, i] (bf16)
  s_loc[b, (c,o)]  = sum_j xpk_j.T @ (wp_j * cex_j)    (cex = exp(b) bcast)
  G_j  [(rl,i),(c,o)] = xt_j.T @ v                      (sum over batch)
  m_j  [rl, c]     = (1/B) * sum_{i,o} wp_j * G_j       (b-logit update)
"""

import sys

if "/opt/trn_rl_repo" not in sys.path:
    sys.path.insert(0, "/opt/trn_rl_repo")

from contextlib import ExitStack

import ml_dtypes
import numpy as np

import concourse.bass as bass
import concourse.tile as tile
from concourse import bacc, bass_utils, mybir

B, R, C, O, I = 128, 4608, 16, 16, 8
NCORES = 8
RL = R // NCORES          # 576 routes per core
RBLK = 16                 # routes per 128-partition block
NBLK = RL // RBLK         # 36
CO = C * O                # 256
ITERS = 3
F32 = mybir.dt.float32
BF16 = mybir.dt.bfloat16


def _bc(ap, counts):
    """View `ap` ([P, n] free layout) broadcast along an appended 0-step dim."""
    return bass.AP(tensor=ap.tensor, offset=ap.offset, ap=list(ap.ap) + [[0, counts]])


def _build(nc, reps=1, chain=False, ar_only=False, no_cc=False):
    f32 = F32
    xt_d = nc.dram_tensor("xt", [B, RL * I], BF16, kind="ExternalInput").ap()
    wp_d = nc.dram_tensor("wp", [NBLK, 128, CO], BF16, kind="ExternalInput").ap()
    emat_d = nc.dram_tensor("emat", [RBLK, 128], f32, kind="ExternalInput").ap()
    ematT_d = nc.dram_tensor("ematT", [128, RBLK], f32, kind="ExternalInput").ap()
    ones1_d = nc.dram_tensor("ones1", [1, 128], f32, kind="ExternalInput").ap()
    ones16_d = nc.dram_tensor("ones16", [RBLK, 1], f32, kind="ExternalInput").ap()
    out_d = nc.dram_tensor("out", [B, C, O, 1], f32, kind="ExternalOutput").ap()

    NAR = B * CO + C  # AllReduce payload: s_unnorm then D
    cc_in = [nc.dram_tensor(f"cc_in{i}", [NAR], f32) for i in range(ITERS * reps)]
    cc_out = [
        nc.dram_tensor(f"cc_out{i}", [NAR], f32, addr_space="Shared")
        for i in range(ITERS * reps)
    ]
    groups = [list(range(NCORES))]

    with tile.TileContext(nc) as tc, ExitStack() as ctx:
        const = ctx.enter_context(tc.tile_pool(name="const", bufs=1))
        wpool = ctx.enter_context(tc.tile_pool(name="wpool", bufs=1))
        xpool = ctx.enter_context(tc.tile_pool(name="xpool", bufs=1))
        work = ctx.enter_context(tc.tile_pool(name="work", bufs=3))
        sq_pool = ctx.enter_context(tc.tile_pool(name="sq", bufs=2))
        ps_s = ctx.enter_context(tc.tile_pool(name="ps_s", bufs=1, space="PSUM"))
        ps_big = ctx.enter_context(tc.tile_pool(name="ps_big", bufs=2, space="PSUM"))
        ps_cex = ctx.enter_context(tc.tile_pool(name="ps_cex", bufs=2, space="PSUM"))
        ps_m = ctx.enter_context(tc.tile_pool(name="ps_m", bufs=1, space="PSUM"))
        ps_d = ctx.enter_context(tc.tile_pool(name="ps_d", bufs=1, space="PSUM"))

        dma = nc.sync.dma_start

        emat_sb = const.tile([RBLK, 128], f32, tag="emat")
        dma(out=emat_sb, in_=emat_d)
        ematT_sb = const.tile([128, RBLK], f32, tag="ematT")
        dma(out=ematT_sb, in_=ematT_d)
        ones1_sb = const.tile([1, 128], f32, tag="ones1")
        dma(out=ones1_sb, in_=ones1_d)
        ones16_sb = const.tile([RBLK, 1], f32, tag="ones16")
        dma(out=ones16_sb, in_=ones16_d)
        dinit_sb = const.tile([1, C], f32, tag="dinit")
        nc.vector.memset(dinit_sb, float(RL))

        xt_sb = xpool.tile([B, RL * I], BF16, tag="xt")
        dma(out=xt_sb, in_=xt_d)

        wp_sb = []
        for j in range(NBLK):
            w = wpool.tile([128, CO], BF16, tag=f"wp{j}")
            dma(out=w, in_=wp_d[j])
            wp_sb.append(w)

        # xpk_j = xt block transposed -> [(rl,i), b], via DMA-transpose from HBM
        xpk_sb = []
        for j in range(NBLK):
            xk = xpool.tile([128, 128], BF16, tag=f"xpk{j}")
            nc.sync.dma_start_transpose(out=xk, in_=xt_d[:, j * 128:(j + 1) * 128])
            xpk_sb.append(xk)

        # routing logits, core-local: [16 (rl), NBLK, C]
        b_sb = const.tile([RBLK, NBLK, C], f32, tag="b")
        if chain:
            nc.vector.memset(b_sb, 0.0)

        if ar_only:
            # isolate collective cost: 3*reps chained AllReduces, no compute
            seed = sq_pool.tile([B, CO], f32, tag="s_sb")
            nc.vector.memset(seed, 1.0)
            dma(out=cc_in[0].ap()[0:B * CO].rearrange("(b n) -> b n", b=B), in_=seed)
            for k in range(ITERS * reps):
                nc.gpsimd.collective_compute(
                    "AllReduce", mybir.AluOpType.add, replica_groups=groups,
                    ins=[cc_in[k].ap()], outs=[cc_out[k].ap()],
                )
                if k + 1 < ITERS * reps:
                    t = sq_pool.tile([B, CO], f32, tag="s_sb")
                    dma(out=t, in_=cc_out[k].ap()[0:B * CO].rearrange(
                        "(b n) -> b n", b=B))
                    dma(out=cc_in[k + 1].ap()[0:B * CO].rearrange(
                        "(b n) -> b n", b=B), in_=t)
            v_sb = sq_pool.tile([B, C, O], f32, tag="v")
            dma(out=v_sb,
                in_=cc_out[ITERS * reps - 1].ap()[0:B * CO].rearrange(
                    "(b c o) -> b c o", b=B, c=C))
        else:
            v_sb = None
            for rep in range(reps):
                v_sb = _routing(
                    nc, tc, rep, wp_sb, xpk_sb, xt_sb, b_sb, emat_sb, ematT_sb,
                    ones1_sb, ones16_sb, dinit_sb, cc_in, cc_out, groups,
                    work, sq_pool, ps_s, ps_big, ps_cex, ps_m, ps_d, dma,
                    chain=chain, no_cc=no_cc,
                )

        # --- output: v [128, 256] -> [128, 16, 16, 1] ---
        dma(out=out_d.rearrange("b c o a -> b (c o a)"), in_=v_sb)

    nc.compile()
    return nc


def _routing(
    nc, tc, rep, wp_sb, xpk_sb, xt_sb, b_sb, emat_sb, ematT_sb,
    ones1_sb, ones16_sb, dinit_sb, cc_in, cc_out, groups,
    work, sq_pool, ps_s, ps_big, ps_cex, ps_m, ps_d, dma,
    chain=False, no_cc=False,
):
    f32 = F32
    NAR = B * CO + C
    if True:
        v_sb = None
        for it in range(ITERS):
            weighted = chain or it > 0
            # --- route weights exp(b) (skipped for it0: uniform) ---
            if weighted:
                ebx = work.tile([RBLK, NBLK, C], f32, tag="ebx")
                nc.scalar.activation(
                    out=ebx, in_=b_sb, func=mybir.ActivationFunctionType.Exp
                )
                # local softmax denom: sum over (rl, blk) -> [1, C]
                dpart = work.tile([RBLK, C], f32, tag="dpart")
                nc.vector.reduce_sum(
                    out=dpart,
                    in_=ebx.rearrange("p n c -> p c n"),
                    axis=mybir.AxisListType.X,
                )
                dloc_ps = ps_d.tile([1, C], f32, tag="dloc")
                nc.tensor.matmul(dloc_ps, lhsT=ones16_sb, rhs=dpart,
                                 start=True, stop=True)
                dloc_sb = work.tile([1, C], f32, tag="dloc_sb")
                nc.vector.tensor_copy(out=dloc_sb, in_=dloc_ps)

            # --- s-pass: s_loc[b,(c,o)] = sum_j xpk_j.T @ wc_j ---
            s_ps = ps_s.tile([B, CO], f32, tag="s")
            for j in range(NBLK):
                if not weighted:
                    rhs = wp_sb[j]
                else:
                    cex_ps = ps_cex.tile([128, C], f32, tag="cex")
                    nc.tensor.matmul(
                        cex_ps, lhsT=emat_sb, rhs=ebx[:, j, :], start=True, stop=True
                    )
                    wc = work.tile([128, C, O], BF16, tag="wc")
                    nc.vector.tensor_tensor(
                        out=wc,
                        in0=wp_sb[j].rearrange("p (c o) -> p c o", o=O),
                        in1=_bc(cex_ps, O),
                        op=mybir.AluOpType.mult,
                    )
                    rhs = wc.rearrange("p c o -> p (c o)")
                nc.tensor.matmul(
                    s_ps,
                    lhsT=xpk_sb[j],
                    rhs=rhs,
                    start=(j == 0),
                    stop=(j == NBLK - 1),
                )

            # --- fused AllReduce: [s_unnorm (32768) | D (16)] ---
            ci, co_ = cc_in[rep * ITERS + it].ap(), cc_out[rep * ITERS + it].ap()
            s_loc_sb = sq_pool.tile([B, CO], f32, tag="s_loc")
            nc.vector.tensor_copy(out=s_loc_sb, in_=s_ps)
            dma(out=ci[0:B * CO].rearrange("(b n) -> b n", b=B), in_=s_loc_sb)
            if not weighted:
                dma(out=ci[B * CO:NAR].rearrange("(a c) -> a c", a=1), in_=dinit_sb)
            else:
                dma(out=ci[B * CO:NAR].rearrange("(a c) -> a c", a=1), in_=dloc_sb)
            if no_cc:
                co_ = ci  # cost-model variant: skip the collective
            else:
                nc.gpsimd.collective_compute(
                    "AllReduce",
                    mybir.AluOpType.add,
                    replica_groups=groups,
                    ins=[ci],
                    outs=[co_],
                )
            s_sb = sq_pool.tile([B, CO], f32, tag="s_sb")
            dma(out=s_sb, in_=co_[0:B * CO].rearrange("(b n) -> b n", b=B))
            dg_sb = sq_pool.tile([1, C], f32, tag="dg")
            dma(out=dg_sb, in_=co_[B * CO:NAR].rearrange("(a c) -> a c", a=1))

            # --- normalize by D (bcast to all partitions via 1-row matmul) ---
            dfull_ps = ps_d.tile([B, C], f32, tag="dfull")
            nc.tensor.matmul(dfull_ps, lhsT=ones1_sb, rhs=dg_sb,
                             start=True, stop=True)
            dr = sq_pool.tile([B, C], f32, tag="dr")
            nc.vector.reciprocal(out=dr, in_=dfull_ps)
            sN = sq_pool.tile([B, C, O], f32, tag="sN")
            nc.vector.tensor_tensor(
                out=sN,
                in0=s_sb.rearrange("b (c o) -> b c o", o=O),
                in1=_bc(dr, O),
                op=mybir.AluOpType.mult,
            )

            # --- squash ---
            sqs = sq_pool.tile([B, C, O], f32, tag="sqs")
            nc.vector.tensor_mul(out=sqs, in0=sN, in1=sN)
            sn = sq_pool.tile([B, C], f32, tag="sn")
            nc.vector.reduce_sum(out=sn, in_=sqs, axis=mybir.AxisListType.X)
            st = sq_pool.tile([B, C], f32, tag="st")
            nc.scalar.sqrt(out=st, in_=sn)
            w1 = sq_pool.tile([B, C], f32, tag="w1")
            nc.vector.tensor_mul(out=w1, in0=sn, in1=st)    # sn^1.5
            nc.vector.tensor_add(out=w1, in0=w1, in1=st)    # (1+sn)*sqrt(sn)
            rc = sq_pool.tile([B, C], f32, tag="rc")
            nc.vector.reciprocal(out=rc, in_=w1)
            fct = sq_pool.tile([B, C], f32, tag="fct")
            nc.vector.tensor_mul(out=fct, in0=sn, in1=rc)   # sn/((1+sn)sqrt(sn))
            v_sb = sq_pool.tile([B, C, O], f32, tag="v")
            nc.vector.tensor_tensor(
                out=v_sb, in0=sN, in1=_bc(fct, O), op=mybir.AluOpType.mult
            )

            # --- a-pass + b update (not needed after last iteration) ---
            if it < ITERS - 1:
                v_bf = sq_pool.tile([B, CO], BF16, tag="v_bf")
                nc.vector.tensor_copy(
                    out=v_bf, in_=v_sb.rearrange("b c o -> b (c o)")
                )
                for j in range(NBLK):
                    g_ps = ps_big.tile([B, CO], f32, tag="big")
                    nc.tensor.matmul(
                        g_ps,
                        lhsT=xt_sb[:, j * 128:(j + 1) * 128],
                        rhs=v_bf,
                        start=True,
                        stop=True,
                    )
                    pt = work.tile([128, C, O], f32, tag="pt")
                    nc.vector.tensor_mul(
                        out=pt,
                        in0=wp_sb[j].rearrange("p (c o) -> p c o", o=O),
                        in1=g_ps.rearrange("p (c o) -> p c o", o=O),
                    )
                    tr = work.tile([128, C], f32, tag="tr")
                    nc.vector.reduce_sum(out=tr, in_=pt, axis=mybir.AxisListType.X)
                    m_ps = ps_m.tile([RBLK, C], f32, tag="m")
                    nc.tensor.matmul(m_ps, lhsT=ematT_sb, rhs=tr,
                                     start=True, stop=True)
                    if it == 0:
                        nc.vector.tensor_copy(out=b_sb[:, j, :], in_=m_ps)
                    else:
                        nc.vector.tensor_add(
                            out=b_sb[:, j, :], in0=b_sb[:, j, :], in1=m_ps
                        )
        return v_sb


_NC_CACHE = None


def _get_nc():
    global _NC_CACHE
    if _NC_CACHE is None:
        nc = bacc.Bacc(
            "TRN2", target_bir_lowering=False, debug=False, num_devices=NCORES
        )
        _NC_CACHE = _build(nc)
    return _NC_CACHE


def make_in_maps(x, W):
    x = np.ascontiguousarray(np.asarray(x, np.float32))
    W = np.ascontiguousarray(np.asarray(W, np.float32))
    emat = np.zeros((RBLK, 128), np.float32)
    for rl in range(RBLK):
        emat[rl, rl * I:(rl + 1) * I] = 1.0
    ematT = np.ascontiguousarray(emat.T) / np.float32(B)
    ones1 = np.ones((1, 128), np.float32)
    ones16 = np.ones((RBLK, 1), np.float32)
    in_maps = []
    for cid in range(NCORES):
        r0 = cid * RL
        xt = (
            np.ascontiguousarray(x[:, r0:r0 + RL, :])
            .reshape(B, RL * I)
            .astype(ml_dtypes.bfloat16)
        )
        Wl = W[0, r0:r0 + RL]  # [RL, C, O, I]
        wp = (
            np.ascontiguousarray(
                Wl.reshape(NBLK, RBLK, C, O, I).transpose(0, 1, 4, 2, 3)
            )
            .reshape(NBLK, 128, CO)
            .astype(ml_dtypes.bfloat16)
        )
        in_maps.append(
            {
                "xt": xt,
                "wp": wp,
                "emat": emat,
                "ematT": ematT,
                "ones1": ones1,
                "ones16": ones16,
            }
        )
    return in_maps


def kernel(x, W):
    nc = _get_nc()
    in_maps = make_in_maps(x, W)
    res = bass_utils.run_bass_kernel_spmd(nc, in_maps, core_ids=list(range(NCORES)))
    out = np.asarray(res.results[0]["out"], np.float32)
    return out.reshape(B, C, O, 1)


# revision 38
# speedup vs baseline: 1.0054x; 1.0054x over previous
"""DigitCaps dynamic-routing kernel for 8 Trainium2 NeuronCores.

Strategy: shard the ROUTE dimension (4608 -> 576/core) instead of batch.
Per core, the x-shard and the packed W-shard fit in SBUF (bf16: 3.5MB), so
all 5 passes over u_hat (3x s-pass, 2x a-pass) recompute u_hat from
SBUF-resident data via PE matmuls -- no HBM re-streaming.  The only
cross-core traffic is one fused AllReduce per routing iteration carrying
s_unnorm [128,256] + the local softmax denominator [16].  The routing
logits b stay core-local (they are route-indexed).

Precision: matmul operands for the two big passes are bf16 (fp32 PSUM
accumulation); everything else fp32.  Validated end-to-end rel err ~2.6e-3
vs the fp32 reference.

Math mapping (per core, r0 = core*576, 36 blocks of 16 routes):
  xt   [128b, (r,i)=4608]          x[:, r0:r0+576, :] flattened (bf16)
  xpk  [128=(rl,i), 36, 128b]      DMA-transposed xt (bf16)
  wp   [128=(rl,i), 36, 16c, 16o]  W[r0+16j+rl, c, o, i] (bf16)
  s_loc[b, (c,o)]  = sum_j xpk[:,j,:].T @ (wp_j * exp(b_j) bcast over o,i)
  G_j  [(rl,i),(c,o)] = xt_j.T @ v                  (contracts batch)
  m_j  [rl, c]     = (1/B) * sum_{i,o} wp_j * G_j   (b-logit update)

DVE work is batched: one wc multiply per iteration ([128,576,16], bf16 2x),
a-pass multiply/reduce in groups of GBLK=6 blocks through a 3-bank PSUM
tile, cex/m matmuls batched 2 per iteration (N=288).
"""

import sys

if "/opt/trn_rl_repo" not in sys.path:
    sys.path.insert(0, "/opt/trn_rl_repo")

from contextlib import ExitStack

import ml_dtypes
import numpy as np

import concourse.bass as bass
import concourse.tile as tile
from concourse import bacc, bass_utils, mybir

B, R, C, O, I = 128, 4608, 16, 16, 8
NCORES = 8
RL = R // NCORES          # 576 routes per core
RBLK = 16                 # routes per 128-partition block
NBLK = RL // RBLK         # 36
GBLK = 6                  # blocks per a-pass PSUM group (3 banks)
NG = NBLK // GBLK         # 6 groups
CO = C * O                # 256
JC = NBLK * C             # 576 (block, capsule) pairs
ITERS = 3
F32 = mybir.dt.float32
BF16 = mybir.dt.bfloat16


def _bc(ap, counts):
    """View `ap` broadcast along an appended 0-step free dim."""
    return bass.AP(tensor=ap.tensor, offset=ap.offset, ap=list(ap.ap) + [[0, counts]])


def _bc0(ap, counts):
    """View `ap` broadcast along a 0-step dim INSERTED before the free dims.
    Keeps the last dim packed so DVE 2x/4x modes stay eligible."""
    a = list(ap.ap)
    return bass.AP(tensor=ap.tensor, offset=ap.offset,
                   ap=[a[0], [0, counts]] + a[1:])


def _build(nc, reps=1, chain=False, ar_only=False, no_cc=False):
    f32 = F32
    xt_d = nc.dram_tensor("xt", [B, RL * I], BF16, kind="ExternalInput").ap()
    # host-packed, partition-major, o-MAJOR free: wp[p, (o, j, c)] =
    # W[r0+16j+(p>>3), c, o, p&7].  o-major keeps every DVE operand's last
    # dim packed (stride 1) so the wc multiply runs in 2x/4x mode.
    wp_d = nc.dram_tensor("wp", [128, O * NBLK * C], BF16, kind="ExternalInput").ap()
    ematT_d = nc.dram_tensor("ematT", [128, RBLK], f32, kind="ExternalInput").ap()
    emat_d = nc.dram_tensor("emat", [RBLK, 128], BF16, kind="ExternalInput").ap()
    ones1_d = nc.dram_tensor("ones1", [1, 128], f32, kind="ExternalInput").ap()
    ones16_d = nc.dram_tensor("ones16", [RBLK, 1], f32, kind="ExternalInput").ap()
    out_d = nc.dram_tensor("out", [B, C, O, 1], f32, kind="ExternalOutput").ap()

    NAR = B * CO + C  # AllReduce payload: s_unnorm then D
    cc_in = [nc.dram_tensor(f"cc_in{i}", [NAR], f32) for i in range(ITERS * reps)]
    cc_out = [
        nc.dram_tensor(f"cc_out{i}", [NAR], f32, addr_space="Shared")
        for i in range(ITERS * reps)
    ]
    groups = [list(range(NCORES))]

    with tile.TileContext(nc) as tc, ExitStack() as ctx:
        const = ctx.enter_context(tc.tile_pool(name="const", bufs=1))
        xpool = ctx.enter_context(tc.tile_pool(name="xpool", bufs=1))
        work = ctx.enter_context(tc.tile_pool(name="work", bufs=2))
        sq_pool = ctx.enter_context(tc.tile_pool(name="sq", bufs=2))
        ps_s = ctx.enter_context(tc.tile_pool(name="ps_s", bufs=1, space="PSUM"))
        ps_g = ctx.enter_context(tc.tile_pool(name="ps_g", bufs=1, space="PSUM"))
        ps_cex = ctx.enter_context(tc.tile_pool(name="ps_cex", bufs=1, space="PSUM"))
        ps_m = ctx.enter_context(tc.tile_pool(name="ps_m", bufs=1, space="PSUM"))
        ps_d = ctx.enter_context(tc.tile_pool(name="ps_d", bufs=1, space="PSUM"))

        dma = nc.sync.dma_start

        # xpk[p, j, b] = xt[b, 128j+p] -- single 3D DMA-transpose. Issued
        # FIRST: it gates the it0 s-pass.
        xpk = xpool.tile([128, NBLK, B], BF16, tag="xpk")
        nc.sync.dma_start_transpose(out=xpk, in_=xt_d)

        # wp_all[p, (o, j, c)] -- loaded as two jc-halves on the gpsimd
        # queue so the it0 s-pass of half 0 can start during half 1's load.
        wp_all = xpool.tile([128, O * NBLK * C], BF16, tag="wp")
        wp3 = wp_all.rearrange("p (o jc) -> p o jc", jc=JC)
        wp3_d = wp_d.rearrange("p (o jc) -> p o jc", jc=JC)
        H = JC // 2
        nc.gpsimd.dma_start(out=wp3[:, :, 0:H], in_=wp3_d[:, :, 0:H])
        nc.gpsimd.dma_start(out=wp3[:, :, H:JC], in_=wp3_d[:, :, H:JC])

        xt_sb = xpool.tile([B, RL * I], BF16, tag="xt")
        dma(out=xt_sb, in_=xt_d)

        emat_sb = const.tile([RBLK, 128], BF16, tag="emat")
        dma(out=emat_sb, in_=emat_d)
        ematT_sb = const.tile([128, RBLK], f32, tag="ematT")
        dma(out=ematT_sb, in_=ematT_d)
        ones1_sb = const.tile([1, 128], f32, tag="ones1")
        dma(out=ones1_sb, in_=ones1_d)
        ones16_sb = const.tile([RBLK, 1], f32, tag="ones16")
        dma(out=ones16_sb, in_=ones16_d)
        dinit_sb = const.tile([1, C], f32, tag="dinit")
        nc.vector.memset(dinit_sb, float(RL))

        # routing logits, core-local, split in two 18-block halves for
        # fine-grained pipelining: [16 (rl), 18, C] each
        b_halves = [
            const.tile([RBLK, NBLK // 2, C], f32, tag=f"b{h}", name=f"b{h}")
            for h in range(2)
        ]
        if chain:
            for bh in b_halves:
                nc.vector.memset(bh, 0.0)

        if ar_only:
            # isolate collective cost: 3*reps chained AllReduces, no compute
            seed = sq_pool.tile([B, CO], f32, tag="s_sb")
            nc.vector.memset(seed, 1.0)
            dma(out=cc_in[0].ap()[0:B * CO].rearrange("(b n) -> b n", b=B), in_=seed)
            for k in range(ITERS * reps):
                nc.gpsimd.collective_compute(
                    "AllReduce", mybir.AluOpType.add, replica_groups=groups,
                    ins=[cc_in[k].ap()], outs=[cc_out[k].ap()],
                )
                if k + 1 < ITERS * reps:
                    t = sq_pool.tile([B, CO], f32, tag="s_sb")
                    dma(out=t, in_=cc_out[k].ap()[0:B * CO].rearrange(
                        "(b n) -> b n", b=B))
                    dma(out=cc_in[k + 1].ap()[0:B * CO].rearrange(
                        "(b n) -> b n", b=B), in_=t)
            v_sb = sq_pool.tile([B, C, O], f32, tag="v")
            dma(out=v_sb,
                in_=cc_out[ITERS * reps - 1].ap()[0:B * CO].rearrange(
                    "(b c o) -> b c o", b=B, c=C))
        else:
            v_sb = None
            for rep in range(reps):
                v_sb = _routing(
                    nc, rep, wp_all, xpk, xt_sb, b_halves, emat_sb, ematT_sb,
                    ones1_sb, ones16_sb, dinit_sb, cc_in, cc_out, groups,
                    work, sq_pool, ps_s, ps_g, ps_cex, ps_m, ps_d, dma,
                    chain=chain, no_cc=no_cc,
                )

        # --- output: v [128, 256] -> [128, 16, 16, 1] ---
        dma(out=out_d.rearrange("b c o a -> b (c o a)"), in_=v_sb)

    nc.compile()
    return nc


def _routing(
    nc, rep, wp_all, xpk, xt_sb, b_halves, emat_sb, ematT_sb,
    ones1_sb, ones16_sb, dinit_sb, cc_in, cc_out, groups,
    work, sq_pool, ps_s, ps_g, ps_cex, ps_m, ps_d, dma,
    chain=False, no_cc=False,
):
    f32 = F32
    NAR = B * CO + C
    HJ = NBLK // 2            # 18 blocks per half
    HJC = HJ * C              # 288
    HG = HJ // GBLK           # 3 a-pass groups per half
    # views of the o-major packed W: [p, o, (j,c)] and c-major [p, (j,c), o]
    wp_ojc = wp_all.rearrange("p (o jc) -> p o jc", jc=JC)      # [128, 16, 576]
    wp_cmaj = wp_all.rearrange("p (o jc) -> p jc o", jc=JC)     # [128, 576, 16]

    def prep_half(h):
        """exp(b) -> cex -> wc for blocks [h*18, h*18+18).  Separate tiles per
        half so Tile's dependency tracking lets the next s-pass half start
        while the other half is still in the a-pass."""
        ebx = work.tile([RBLK, HJ, C], BF16, tag=f"ebx{h}")
        nc.scalar.activation(
            out=ebx, in_=b_halves[h], func=mybir.ActivationFunctionType.Exp
        )
        dpart = work.tile([RBLK, C], f32, tag=f"dpart{h}")
        nc.vector.reduce_sum(
            out=dpart, in_=ebx.rearrange("p n c -> p c n"),
            axis=mybir.AxisListType.X,
        )
        cex_ps = ps_cex.tile([128, HJC], f32, tag="cex")
        nc.tensor.matmul(cex_ps, lhsT=emat_sb,
                         rhs=ebx.rearrange("p n c -> p (n c)"),
                         start=True, stop=True)
        cex_sb = work.tile([128, HJC], BF16, tag=f"cex{h}")
        nc.vector.tensor_copy(out=cex_sb, in_=cex_ps)
        wc = work.tile([128, O, HJC], BF16, tag=f"wc{h}")
        nc.vector.tensor_tensor(
            out=wc, in0=wp_ojc[:, :, h * HJC:(h + 1) * HJC],
            in1=_bc0(cex_sb, O), op=mybir.AluOpType.mult,
        )
        return wc.rearrange("p o jc -> p jc o"), dpart

    v_sb = None
    wc_halves = None
    dparts = None
    if chain:
        wc_halves, dparts = zip(prep_half(0), prep_half(1))
    for it in range(ITERS):
        weighted = chain or it > 0

        # --- s-pass: s_loc[b,(c,o)] = sum_j xpk_j.T @ rhs_j ---
        s_ps = ps_s.tile([B, CO], f32, tag="s")
        for j in range(NBLK):
            if weighted:
                rhs = wc_halves[j // HJ][:, (j % HJ) * C:((j % HJ) + 1) * C, :]
            else:
                rhs = wp_cmaj[:, j * C:(j + 1) * C, :]
            nc.tensor.matmul(
                s_ps, lhsT=xpk[:, j, :], rhs=rhs,
                start=(j == 0), stop=(j == NBLK - 1),
            )

        # --- fused AllReduce: [s_unnorm (32768) | D (16)] ---
        ci, co_ = cc_in[rep * ITERS + it].ap(), cc_out[rep * ITERS + it].ap()
        s_loc_sb = sq_pool.tile([B, CO], f32, tag="s_loc")
        nc.vector.tensor_copy(out=s_loc_sb, in_=s_ps)
        dma(out=ci[0:B * CO].rearrange("(b n) -> b n", b=B), in_=s_loc_sb)
        if weighted:
            dadd = work.tile([RBLK, C], f32, tag="dadd")
            nc.vector.tensor_add(out=dadd, in0=dparts[0], in1=dparts[1])
            dloc_ps = ps_d.tile([1, C], f32, tag="d")
            nc.tensor.matmul(dloc_ps, lhsT=ones16_sb, rhs=dadd,
                             start=True, stop=True)
            dloc_sb = work.tile([1, C], f32, tag="dloc_sb")
            nc.vector.tensor_copy(out=dloc_sb, in_=dloc_ps)
            dma(out=ci[B * CO:NAR].rearrange("(a c) -> a c", a=1), in_=dloc_sb)
        else:
            dma(out=ci[B * CO:NAR].rearrange("(a c) -> a c", a=1), in_=dinit_sb)
        if no_cc:
            co_ = ci  # cost-model variant: skip the collective
        else:
            nc.gpsimd.collective_compute(
                "AllReduce", mybir.AluOpType.add, replica_groups=groups,
                ins=[ci], outs=[co_],
            )
        s_sb = sq_pool.tile([B, CO], f32, tag="s_sb")
        dma(out=s_sb, in_=co_[0:B * CO].rearrange("(b n) -> b n", b=B))
        dg_sb = sq_pool.tile([1, C], f32, tag="dg")
        dma(out=dg_sb, in_=co_[B * CO:NAR].rearrange("(a c) -> a c", a=1))

        # --- squash with 1/D folded in:
        #   sn = (sum_o s_raw^2) / D^2;  fct = sn/((1+sn)sqrt(sn))
        #   v  = s_raw * fct / D
        dfull_ps = ps_d.tile([B, C], f32, tag="d")
        nc.tensor.matmul(dfull_ps, lhsT=ones1_sb, rhs=dg_sb, start=True, stop=True)
        dr = sq_pool.tile([B, C], f32, tag="dr")
        nc.vector.reciprocal(out=dr, in_=dfull_ps)
        sqs = sq_pool.tile([B, C, O], f32, tag="sqs")
        nc.vector.tensor_mul(
            out=sqs,
            in0=s_sb.rearrange("b (c o) -> b c o", o=O),
            in1=s_sb.rearrange("b (c o) -> b c o", o=O),
        )
        snr = sq_pool.tile([B, C], f32, tag="snr")
        nc.vector.reduce_sum(out=snr, in_=sqs, axis=mybir.AxisListType.X)
        dr2 = sq_pool.tile([B, C], f32, tag="dr2")
        nc.vector.tensor_mul(out=dr2, in0=dr, in1=dr)
        sn = sq_pool.tile([B, C], f32, tag="sn")
        nc.vector.tensor_mul(out=sn, in0=snr, in1=dr2)       # sn scaled
        st = sq_pool.tile([B, C], f32, tag="st")
        nc.scalar.sqrt(out=st, in_=sn)
        w1 = sq_pool.tile([B, C], f32, tag="w1")
        nc.vector.tensor_mul(out=w1, in0=sn, in1=st)         # sn^1.5
        nc.vector.tensor_add(out=w1, in0=w1, in1=st)         # (1+sn)sqrt(sn)
        rc = sq_pool.tile([B, C], f32, tag="rc")
        nc.vector.reciprocal(out=rc, in_=w1)
        fct = sq_pool.tile([B, C], f32, tag="fct")
        nc.vector.tensor_mul(out=fct, in0=sn, in1=rc)        # sn/((1+sn)sqrt(sn))
        nc.vector.tensor_mul(out=fct, in0=fct, in1=dr)       # ... / D
        v_sb = sq_pool.tile([B, C, O], f32, tag="v")
        nc.vector.tensor_tensor(
            out=v_sb,
            in0=s_sb.rearrange("b (c o) -> b c o", o=O),
            in1=_bc(fct, O),
            op=mybir.AluOpType.mult,
        )

        # --- a-pass + next-iteration prep, pipelined per 18-block half ---
        if it < ITERS - 1:
            v_bf = sq_pool.tile([B, CO], BF16, tag="v_bf")
            nc.vector.tensor_copy(out=v_bf, in_=v_sb.rearrange("b c o -> b (c o)"))
            wc_halves = [None, None]
            dparts = [None, None]
            for h in range(2):
                tr_h = work.tile([128, HJC], f32, tag=f"tr{h}")
                for g in range(HG):
                    g_ps = ps_g.tile([B, GBLK * CO], f32, tag="g")
                    for k in range(GBLK):
                        j = h * HJ + g * GBLK + k
                        nc.tensor.matmul(
                            g_ps[:, k * CO:(k + 1) * CO],
                            lhsT=xt_sb[:, j * 128:(j + 1) * 128],
                            rhs=v_bf,
                            start=True, stop=True,
                        )
                    pt = work.tile([128, GBLK * C, O], f32, tag="pt")
                    base = h * HJC + g * GBLK * C
                    nc.vector.tensor_mul(
                        out=pt,
                        in0=wp_cmaj[:, base:base + GBLK * C, :],
                        in1=g_ps.rearrange("p (jc o) -> p jc o", o=O),
                    )
                    nc.vector.reduce_sum(
                        out=tr_h[:, g * GBLK * C:(g + 1) * GBLK * C],
                        in_=pt, axis=mybir.AxisListType.X,
                    )
                m_ps = ps_m.tile([RBLK, HJC], f32, tag="m")
                nc.tensor.matmul(m_ps, lhsT=ematT_sb, rhs=tr_h,
                                 start=True, stop=True)
                bh = b_halves[h].rearrange("p n c -> p (n c)")
                if it == 0 and not chain:
                    nc.vector.tensor_copy(out=bh, in_=m_ps)
                else:
                    nc.vector.tensor_add(out=bh, in0=bh, in1=m_ps)
                # next-iteration weights for this half, immediately
                wc_halves[h], dparts[h] = prep_half(h)
    return v_sb


_NC_CACHE = None


def _get_nc():
    global _NC_CACHE
    if _NC_CACHE is None:
        nc = bacc.Bacc(
            "TRN2", target_bir_lowering=False, debug=False, num_devices=NCORES
        )
        _NC_CACHE = _build(nc)
    return _NC_CACHE


def make_in_maps(x, W):
    x = np.ascontiguousarray(np.asarray(x, np.float32))
    W = np.ascontiguousarray(np.asarray(W, np.float32))
    emat = np.zeros((RBLK, 128), np.float32)
    for rl in range(RBLK):
        emat[rl, rl * I:(rl + 1) * I] = 1.0
    ematT = np.ascontiguousarray(emat.T) / np.float32(B)
    emat = emat.astype(ml_dtypes.bfloat16)
    ones1 = np.ones((1, 128), np.float32)
    ones16 = np.ones((RBLK, 1), np.float32)
    in_maps = []
    for cid in range(NCORES):
        r0 = cid * RL
        xt = (
            np.ascontiguousarray(x[:, r0:r0 + RL, :])
            .reshape(B, RL * I)
            .astype(ml_dtypes.bfloat16)
        )
        Wl = W[0, r0:r0 + RL]  # [RL, C, O, I]
        # wp[p=(rl,i), (o, j, c)] -- o-major free layout
        wp = (
            np.ascontiguousarray(
                Wl.reshape(NBLK, RBLK, C, O, I).transpose(1, 4, 3, 0, 2)
            )
            .reshape(128, O * NBLK * C)
            .astype(ml_dtypes.bfloat16)
        )
        in_maps.append(
            {
                "xt": xt,
                "wp": wp,
                "emat": emat,
                "ematT": ematT,
                "ones1": ones1,
                "ones16": ones16,
            }
        )
    return in_maps


def kernel(x, W):
    nc = _get_nc()
    in_maps = make_in_maps(x, W)
    res = bass_utils.run_bass_kernel_spmd(nc, in_maps, core_ids=list(range(NCORES)))
    out = np.asarray(res.results[0]["out"], np.float32)
    return out.reshape(B, C, O, 1)
